# revision 6
# baseline (speedup 1.0000x reference)
"""Trainium2 Bass kernel for nn_DomainAttention (moe_routing).

Math (see reference):
    con[n,b]  = cat[n] . x[b]                       # [N, B]
    con      /= max(||con[:,b]||_4, 1e-12)          # 4-norm over N, per column
    p         = softmax(con, axis=N)
    w[s,b]    = sum_{n in chunk s} y[n] * p[n,b]
    theta[s,b]= exp(x[b] . phi[s])
    out[b]    = sigmoid(sum_s w[s,b]*theta[s,b] + bias)

Device strategy (8 cores, data-parallel over B, 512 columns each):
  - con computed as [b_part=128, n_free] tiles: lhsT = x^T chunk (stationary),
    rhs = cat^T chunk (moving), accumulated over the 6 d-chunks in PSUM (bf16
    inputs, fp32 accumulation).  cat^T and x^T stay resident in SBUF.
  - |con|/norm4 <= 1 always (norm4 >= max|con|), so softmax needs no
    max-subtraction: p = e / Z with e = exp(con * inv4), Z = sum e.
  - inv4 = s4^(-1/4) via exponent-shift seed + 2 Newton steps on DVE (no
    Ln/Sqrt on ACT -> single activation table set for the whole kernel).
  - The n axis is permuted on host within each source chunk so y==1 entries
    come first; the masked sum w_s is then just the accumulator of an exp
    activation over the prefix [0:k_s) (k_s baked in at build time), and the
    remainder accumulator gives F_s - w_s.  Z = sum_s F_s on host.
  - Engine budget: PE does the matmuls; PSUM drains (fp32->bf16 con copies)
    split between ACT and DVE; squares on GpSimd; square-accumulate (s4
    partials) on DVE; exp (+prefix accums) on ACT; theta/bias/sigmoid in f64
    on host.
"""

import os

os.environ.setdefault("JAX_PLATFORMS", "axon,cpu")

from contextlib import ExitStack

import ml_dtypes
import numpy as np

import concourse.bass as bass  # noqa: F401
import concourse.tile as tile
from concourse import bacc, bass_utils, mybir

B, D, N, S = 4096, 768, 8192, 4
NCORES = 8
P = 128
BL = B // NCORES          # 512 batch columns per core
NBT = BL // P             # 4 b-tiles per core
NDC = D // P              # 6 contraction chunks
CHUNK = N // S            # 2048 (source chunk along n)
G8 = 1024                 # psum drain chunk along n
NG8 = N // G8             # 8

# How many of the 32 psum drain copies land on ACT (rest go to DVE).
ACT_COPIES = 32
# Magic constant for the y0 ~= x^(-1/4) exponent trick (fast-inverse-sqrt
# style): bits(y0) = K - (bits(x) >> 2).
_QROOT_K = int(round(1.25 * (2 ** 23) * (127 - 0.0450466)))

_F32 = mybir.dt.float32
_BF16 = mybir.dt.bfloat16
_I32 = mybir.dt.int32

_cache: dict = {}


def _emit(ctx, tc, xT, catT, w_out, r_out, ks):
    nc = tc.nc
    AX = mybir.AxisListType.X
    OP = mybir.AluOpType
    AF = mybir.ActivationFunctionType

    cat_pool = ctx.enter_context(tc.tile_pool(name="cat", bufs=24))
    x_pool = ctx.enter_context(tc.tile_pool(name="xp", bufs=1))
    con_pool = ctx.enter_context(tc.tile_pool(name="conp", bufs=NBT))
    sq_pool = ctx.enter_context(tc.tile_pool(name="sqp", bufs=3))
    dmp_pool = ctx.enter_context(tc.tile_pool(name="dmp", bufs=2))
    e_pool = ctx.enter_context(tc.tile_pool(name="ep", bufs=2))
    st_pool = ctx.enter_context(tc.tile_pool(name="st", bufs=1))
    ps_pool = ctx.enter_context(tc.tile_pool(name="ps", bufs=4, space="PSUM"))

    # x^T resident: xT_sb[p, dc*BL + b] = xT[dc*128+p, b]
    xT_sb = x_pool.tile([P, NDC * BL], _BF16, name="xT_sb")
    for dc in range(NDC):
        nc.sync.dma_start(xT_sb[:, dc * BL:(dc + 1) * BL], xT[dc * P:(dc + 1) * P, :])

    # cat^T resident: [128, 2048] per (g4, dc).  The first g4 group is pulled
    # in 512-wide quarters so the first accumulation groups can start sooner.
    cat_sb = {}
    for g4 in range(4):
        for dc in range(NDC):
            ct = cat_pool.tile([P, 2048], _BF16, name=f"cat_{g4}_{dc}", tag="cat")
            cat_sb[(g4, dc)] = ct
    for q in range(4):
        for dc in range(NDC):
            nc.sync.dma_start(
                cat_sb[(0, dc)][:, q * 512:(q + 1) * 512],
                catT[dc * P:(dc + 1) * P, q * 512:(q + 1) * 512],
            )
    for g4 in range(1, 4):
        for dc in range(NDC):
            nc.sync.dma_start(
                cat_sb[(g4, dc)], catT[dc * P:(dc + 1) * P, g4 * 2048:(g4 + 1) * 2048]
            )

    con_sb = [con_pool.tile([P, N], _BF16, name=f"con{bt}", tag="con") for bt in range(NBT)]
    s4p = [st_pool.tile([P, NG8], _F32, name=f"s4p{bt}") for bt in range(NBT)]
    w_sb = st_pool.tile([P, NBT * S], _F32, name="w_sb")
    r_sb = st_pool.tile([P, NBT * S], _F32, name="r_sb")

    drain_idx = [0]

    def mm_chunk(bt, g8):
        """24 matmuls accumulating con[bt, g8*1024:(g8+1)*1024] in PSUM, then
        drain: copy->bf16 con (ACT or DVE), square (GpSimd), sq^2-accum (DVE)."""
        ps = ps_pool.tile([P, G8], _F32, name="ps")
        g4, half = g8 // 2, g8 % 2
        for dc in range(NDC):
            lhsT = xT_sb[:, dc * BL + bt * P: dc * BL + (bt + 1) * P]
            for h in range(2):
                lo = half * 1024 + h * 512
                nc.tensor.matmul(
                    ps[:, h * 512:(h + 1) * 512],
                    lhsT,
                    cat_sb[(g4, dc)][:, lo:lo + 512],
                    start=(dc == 0),
                    stop=(dc == NDC - 1),
                )
        cs = con_sb[bt][:, g8 * G8:(g8 + 1) * G8]
        if drain_idx[0] % 32 < ACT_COPIES:
            nc.scalar.activation(cs, ps, AF.Copy)
        else:
            nc.vector.tensor_copy(cs, ps)
        drain_idx[0] += 1
        sq = sq_pool.tile([P, G8], _BF16, name="sq")
        nc.gpsimd.tensor_tensor(sq, cs, cs, op=OP.mult)
        dmp = dmp_pool.tile([P, G8], _BF16, name="dmp")
        nc.vector.scalar_tensor_tensor(
            out=dmp, in0=sq, scalar=0.0, in1=sq,
            op0=OP.bypass, op1=OP.mult, accum_out=s4p[bt][:, g8:g8 + 1],
        )

    def pass2(bt):
        # s4 = sum of chunk partials; inv4 = s4^(-1/4) via bit trick + Newton.
        s4 = st_pool.tile([P, 1], _F32, name=f"s4_{bt}")
        nc.vector.tensor_reduce(s4, s4p[bt], axis=AX, op=OP.add)
        nc.vector.tensor_scalar_max(s4, s4, 1e-30)
        y = st_pool.tile([P, 1], _F32, name=f"y_{bt}")
        nc.vector.tensor_scalar(y.bitcast(_I32), s4.bitcast(_I32), 2, None,
                                op0=OP.arith_shift_right)
        nc.vector.tensor_scalar(y.bitcast(_I32), y.bitcast(_I32), -1, _QROOT_K,
                                op0=OP.mult, op1=OP.add)
        y2 = st_pool.tile([P, 1], _F32, name=f"y2_{bt}")
        u = st_pool.tile([P, 1], _F32, name=f"u_{bt}")
        for _ in range(2):
            nc.vector.tensor_tensor(y2, y, y, op=OP.mult)
            nc.vector.tensor_tensor(y2, y2, y2, op=OP.mult)      # y^4
            nc.vector.tensor_tensor(u, y2, s4, op=OP.mult)       # a*y^4
            nc.vector.tensor_scalar(u, u, -0.25, 1.25, op0=OP.mult, op1=OP.add)
            nc.vector.tensor_tensor(y, y, u, op=OP.mult)
        # exp over each source chunk; prefix accumulator = w_s, rest = F_s-w_s.
        for s in range(S):
            base = s * CHUNK
            k = ks[s]
            wcol = w_sb[:, bt * S + s: bt * S + s + 1]
            rcol = r_sb[:, bt * S + s: bt * S + s + 1]
            if k > 0:
                e = e_pool.tile([P, CHUNK], _BF16, name="e", tag="e")
                nc.scalar.activation(
                    e[:, :k], con_sb[bt][:, base:base + k], AF.Exp,
                    scale=y, accum_out=wcol,
                )
            else:
                nc.vector.memset(wcol, 0.0)
            if k < CHUNK:
                e2 = e_pool.tile([P, CHUNK], _BF16, name="e2", tag="e")
                nc.scalar.activation(
                    e2[:, :CHUNK - k], con_sb[bt][:, base + k:base + CHUNK], AF.Exp,
                    scale=y, accum_out=rcol,
                )
            else:
                nc.vector.memset(rcol, 0.0)

    for phase in ((0, 1), (2,), (3,)):
        for g8 in range(NG8):
            for bt in phase:
                mm_chunk(bt, g8)
        for bt in phase:
            pass2(bt)

    nc.sync.dma_start(w_out, w_sb)
    nc.sync.dma_start(r_out, r_sb)


def build_program(ks):
    ks = tuple(int(k) for k in ks)
    if ks in _cache:
        return _cache[ks]
    nc = bacc.Bacc("TRN2", target_bir_lowering=False, debug=False, num_devices=NCORES)
    xT = nc.dram_tensor("xTl", [D, BL], _BF16, kind="ExternalInput").ap()
    catT = nc.dram_tensor("catTp", [D, N], _BF16, kind="ExternalInput").ap()
    w_out = nc.dram_tensor("w_out", [P, NBT * S], _F32, kind="ExternalOutput").ap()
    r_out = nc.dram_tensor("r_out", [P, NBT * S], _F32, kind="ExternalOutput").ap()
    with tile.TileContext(nc) as tc, ExitStack() as ctx:
        _emit(ctx, tc, xT, catT, w_out, r_out, ks)
    nc.compile()
    _cache[ks] = nc
    return nc


def host_prep(batch_x, cat, y):
    """Permute n within each source chunk (y==1 first), build bf16 transposed
    inputs. Returns (catT_bf16 [D,N], xT_bf16 [D,B], ks)."""
    y = np.asarray(y)
    perm = np.empty(N, dtype=np.int64)
    ks = []
    for s in range(S):
        ys = y[s * CHUNK:(s + 1) * CHUNK]
        order = np.argsort(ys == 0, kind="stable")  # nonzero first
        perm[s * CHUNK:(s + 1) * CHUNK] = s * CHUNK + order
        ks.append(int((ys != 0).sum()))
    catp = np.asarray(cat)[perm]
    catT = np.ascontiguousarray(catp.T).astype(ml_dtypes.bfloat16)
    xT = np.ascontiguousarray(np.asarray(batch_x).T).astype(ml_dtypes.bfloat16)
    return catT, xT, ks


def host_epilogue(results, batch_x, phi, bias):
    """results: list over cores of {'w_out': [128,16], 'r_out': [128,16]}."""
    theta = np.exp(np.asarray(batch_x, np.float64) @ np.asarray(phi, np.float64).T)
    out = np.empty(B, np.float64)
    for c in range(NCORES):
        w = np.asarray(results[c]["w_out"], np.float64)
        f = w + np.asarray(results[c]["r_out"], np.float64)
        for bt in range(NBT):
            cols = slice(bt * S, (bt + 1) * S)
            z = f[:, cols].sum(axis=1)
            bidx = c * BL + bt * P + np.arange(P)
            out[bidx] = ((w[:, cols] / z[:, None]) * theta[bidx, :]).sum(axis=1)
    out = out + float(np.asarray(bias).reshape(-1)[0])
    return (1.0 / (1.0 + np.exp(-out))).astype(np.float32)


def make_in_maps(catT, xT):
    return [
        {
            "catTp": catT,
            "xTl": np.ascontiguousarray(xT[:, c * BL:(c + 1) * BL]),
        }
        for c in range(NCORES)
    ]


def kernel(batch_x, cat, y, phi, bias):
    catT, xT, ks = host_prep(batch_x, cat, y)
    nc = build_program(ks)
    res = bass_utils.run_bass_kernel_spmd(nc, make_in_maps(catT, xT), core_ids=list(range(NCORES)))
    return host_epilogue(res.results, batch_x, phi, bias)


# revision 10
# speedup vs baseline: 1.1059x; 1.1059x over previous
"""Trainium2 Bass kernel for nn_DomainAttention (moe_routing).

Math (see reference):
    con[n,b]  = cat[n] . x[b]                       # [N, B]
    con      /= max(||con[:,b]||_4, 1e-12)          # 4-norm over N, per column
    p         = softmax(con, axis=N)
    w[s,b]    = sum_{n in chunk s} y[n] * p[n,b]
    theta[s,b]= exp(x[b] . phi[s])
    out[b]    = sigmoid(sum_s w[s,b]*theta[s,b] + bias)

Device strategy (8 cores, data-parallel over B, 512 columns each):
  - con computed as [b_part=128, n_free] tiles: lhsT = x^T chunk (stationary),
    rhs = cat^T chunk (moving), accumulated over the 6 d-chunks in PSUM (bf16
    inputs, fp32 accumulation).  cat^T and x^T stay resident in SBUF.
  - |con|/norm4 <= 1 always (norm4 >= max|con|), so softmax needs no
    max-subtraction: p = e / Z with e = exp(con * inv4), Z = sum e.
  - inv4 = s4^(-1/4) via exponent-shift seed + 2 Newton steps on DVE (no
    Ln/Sqrt on ACT -> single activation table set for the whole kernel).
  - The n axis is permuted on host within each source chunk so y==1 entries
    come first; the masked sum w_s is then just the accumulator of an exp
    activation over the prefix [0:k_s) (k_s baked in at build time), and the
    remainder accumulator gives F_s - w_s.  Z = sum_s F_s on host.
  - Engine budget: PE does the matmuls; PSUM drains (fp32->bf16 con copies)
    split between ACT and DVE; squares on GpSimd; square-accumulate (s4
    partials) on DVE; exp (+prefix accums) on ACT; theta/bias/sigmoid in f64
    on host.
"""

import os

os.environ.setdefault("JAX_PLATFORMS", "axon,cpu")

from contextlib import ExitStack

import ml_dtypes
import numpy as np

import concourse.bass as bass  # noqa: F401
import concourse.tile as tile
from concourse import bacc, bass_utils, mybir

B, D, N, S = 4096, 768, 8192, 4
NCORES = 8
P = 128
BL = B // NCORES          # 512 batch columns per core
NBT = BL // P             # 4 b-tiles per core
NDC = D // P              # 6 contraction chunks
CHUNK = N // S            # 2048 (source chunk along n)
G8 = 1024                 # psum drain chunk along n
NG8 = N // G8             # 8

# How many of the 32 psum drain copies land on ACT (rest go to DVE).
ACT_COPIES = 32
# Magic constant for the y0 ~= x^(-1/4) exponent trick (fast-inverse-sqrt
# style): bits(y0) = K - (bits(x) >> 2).
_QROOT_K = int(round(1.25 * (2 ** 23) * (127 - 0.0450466)))

_F32 = mybir.dt.float32
_BF16 = mybir.dt.bfloat16
_I32 = mybir.dt.int32

_cache: dict = {}


def _emit(ctx, tc, xT, catT, w_out, r_out, ks):
    nc = tc.nc
    AX = mybir.AxisListType.X
    OP = mybir.AluOpType
    AF = mybir.ActivationFunctionType

    cat_pool = ctx.enter_context(tc.tile_pool(name="cat", bufs=24))
    x_pool = ctx.enter_context(tc.tile_pool(name="xp", bufs=1))
    con_pool = ctx.enter_context(tc.tile_pool(name="conp", bufs=NBT))
    sq_pool = ctx.enter_context(tc.tile_pool(name="sqp", bufs=3))
    dmp_pool = ctx.enter_context(tc.tile_pool(name="dmp", bufs=2))
    e_pool = ctx.enter_context(tc.tile_pool(name="ep", bufs=2))
    st_pool = ctx.enter_context(tc.tile_pool(name="st", bufs=1))
    ps_pool = ctx.enter_context(tc.tile_pool(name="ps", bufs=4, space="PSUM"))

    # x^T resident: xT_sb[p, dc*BL + b] = xT[dc*128+p, b]
    xT_sb = x_pool.tile([P, NDC * BL], _BF16, name="xT_sb")
    for dc in range(NDC):
        nc.sync.dma_start(xT_sb[:, dc * BL:(dc + 1) * BL], xT[dc * P:(dc + 1) * P, :])

    # cat^T resident: [128, 2048] per (g4, dc).  The first g4 group is pulled
    # in 512-wide quarters so the first accumulation groups can start sooner.
    cat_sb = {}
    for g4 in range(4):
        for dc in range(NDC):
            ct = cat_pool.tile([P, 2048], _BF16, name=f"cat_{g4}_{dc}", tag="cat")
            cat_sb[(g4, dc)] = ct
    for q in range(4):
        for dc in range(NDC):
            nc.sync.dma_start(
                cat_sb[(0, dc)][:, q * 512:(q + 1) * 512],
                catT[dc * P:(dc + 1) * P, q * 512:(q + 1) * 512],
            )
    for g4 in range(1, 4):
        for dc in range(NDC):
            nc.sync.dma_start(
                cat_sb[(g4, dc)], catT[dc * P:(dc + 1) * P, g4 * 2048:(g4 + 1) * 2048]
            )

    con_sb = [con_pool.tile([P, N], _BF16, name=f"con{bt}", tag="con") for bt in range(NBT)]
    s4p = [st_pool.tile([P, NG8], _F32, name=f"s4p{bt}") for bt in range(NBT)]
    w_sb = st_pool.tile([P, NBT * S], _F32, name="w_sb")
    r_sb = st_pool.tile([P, NBT * S], _F32, name="r_sb")

    drain_idx = [0]

    def mm_chunk(bt, g8):
        """24 matmuls accumulating con[bt, g8*1024:(g8+1)*1024] in PSUM, then
        drain: copy->bf16 con (ACT or DVE), square (GpSimd), sq^2-accum (DVE)."""
        ps = ps_pool.tile([P, G8], _F32, name="ps")
        g4, half = g8 // 2, g8 % 2
        for dc in range(NDC):
            lhsT = xT_sb[:, dc * BL + bt * P: dc * BL + (bt + 1) * P]
            for h in range(2):
                lo = half * 1024 + h * 512
                nc.tensor.matmul(
                    ps[:, h * 512:(h + 1) * 512],
                    lhsT,
                    cat_sb[(g4, dc)][:, lo:lo + 512],
                    start=(dc == 0),
                    stop=(dc == NDC - 1),
                )
        cs = con_sb[bt][:, g8 * G8:(g8 + 1) * G8]
        nc.scalar.activation(cs, ps, AF.Copy)
        drain_idx[0] += 1
        sq = sq_pool.tile([P, G8], _BF16, name="sq")
        # Early chunks' squares go to the otherwise-idle GpSimd (off the s4
        # critical path); late chunks (which gate s4 -> inv4 -> exp) stay on
        # the faster DVE.
        if g8 < NG8 // 2:
            nc.gpsimd.tensor_tensor(sq, cs, cs, op=OP.mult)
        else:
            nc.vector.scalar_tensor_tensor(
                out=sq, in0=cs, scalar=0.0, in1=cs, op0=OP.bypass, op1=OP.mult,
            )
        dmp = dmp_pool.tile([P, G8], _BF16, name="dmp")
        nc.vector.scalar_tensor_tensor(
            out=dmp, in0=sq, scalar=0.0, in1=sq,
            op0=OP.bypass, op1=OP.mult, accum_out=s4p[bt][:, g8:g8 + 1],
        )

    def newton_closure(bt):
        def emit():
            # s4 = sum of chunk partials; inv4 = s4^(-1/4) via bit trick + Newton.
            s4 = st_pool.tile([P, 1], _F32, name=f"s4_{bt}")
            nc.vector.tensor_reduce(s4, s4p[bt], axis=AX, op=OP.add)
            nc.vector.tensor_scalar_max(s4, s4, 1e-30)
            y = st_pool.tile([P, 1], _F32, name=f"y_{bt}")
            nc.vector.tensor_scalar(y.bitcast(_I32), s4.bitcast(_I32), 2, None,
                                    op0=OP.arith_shift_right)
            nc.vector.tensor_scalar(y.bitcast(_I32), y.bitcast(_I32), -1, _QROOT_K,
                                    op0=OP.mult, op1=OP.add)
            y2 = st_pool.tile([P, 1], _F32, name=f"y2_{bt}")
            u = st_pool.tile([P, 1], _F32, name=f"u_{bt}")
            for _ in range(2):
                nc.vector.tensor_tensor(y2, y, y, op=OP.mult)
                nc.vector.tensor_tensor(y2, y2, y2, op=OP.mult)      # y^4
                nc.vector.tensor_tensor(u, y2, s4, op=OP.mult)       # a*y^4
                nc.vector.tensor_scalar(u, u, -0.25, 1.25, op0=OP.mult, op1=OP.add)
                nc.vector.tensor_tensor(y, y, u, op=OP.mult)
            inv4[bt] = y
        return emit

    def exp_closure(bt, s):
        def emit():
            # exp over source chunk s; prefix accumulator = w_s, rest = F_s-w_s.
            base = s * CHUNK
            k = ks[s]
            y = inv4[bt]
            wcol = w_sb[:, bt * S + s: bt * S + s + 1]
            rcol = r_sb[:, bt * S + s: bt * S + s + 1]
            if k > 0:
                e = e_pool.tile([P, CHUNK], _BF16, name="e", tag="e")
                nc.scalar.activation(
                    e[:, :k], con_sb[bt][:, base:base + k], AF.Exp,
                    scale=y, accum_out=wcol,
                )
            else:
                nc.vector.memset(wcol, 0.0)
            if k < CHUNK:
                e2 = e_pool.tile([P, CHUNK], _BF16, name="e2", tag="e")
                nc.scalar.activation(
                    e2[:, :CHUNK - k], con_sb[bt][:, base + k:base + CHUNK], AF.Exp,
                    scale=y, accum_out=rcol,
                )
            else:
                nc.vector.memset(rcol, 0.0)
        return emit

    inv4 = {}
    # Emission order == engine FIFO order (Tile schedules by program order).
    # pass2 work of a phase is spliced into the NEXT phase's chunk slots so
    # exp ops never sit in ACT's FIFO ahead of psum-drain copies whose psum
    # slots the TensorEngine is waiting to reuse.
    pending = []
    for phase in ((0, 1), (2,), (3,)):
        per_slot = -(-len(pending) // (NG8 - 1)) if pending else 0
        for g8 in range(NG8):
            for bt in phase:
                mm_chunk(bt, g8)
            if g8 >= 1:
                for _ in range(per_slot):
                    if pending:
                        pending.pop(0)()
        for bt in phase:
            pending.append(newton_closure(bt))
            for s in range(S):
                pending.append(exp_closure(bt, s))
    for fn in pending:
        fn()

    nc.sync.dma_start(w_out, w_sb)
    nc.sync.dma_start(r_out, r_sb)


def build_program(ks):
    ks = tuple(int(k) for k in ks)
    if ks in _cache:
        return _cache[ks]
    nc = bacc.Bacc("TRN2", target_bir_lowering=False, debug=False, num_devices=NCORES)
    xT = nc.dram_tensor("xTl", [D, BL], _BF16, kind="ExternalInput").ap()
    catT = nc.dram_tensor("catTp", [D, N], _BF16, kind="ExternalInput").ap()
    w_out = nc.dram_tensor("w_out", [P, NBT * S], _F32, kind="ExternalOutput").ap()
    r_out = nc.dram_tensor("r_out", [P, NBT * S], _F32, kind="ExternalOutput").ap()
    with tile.TileContext(nc) as tc, ExitStack() as ctx:
        _emit(ctx, tc, xT, catT, w_out, r_out, ks)
    nc.compile()
    _cache[ks] = nc
    return nc


def host_prep(batch_x, cat, y):
    """Permute n within each source chunk (y==1 first), build bf16 transposed
    inputs. Returns (catT_bf16 [D,N], xT_bf16 [D,B], ks)."""
    y = np.asarray(y)
    perm = np.empty(N, dtype=np.int64)
    ks = []
    for s in range(S):
        ys = y[s * CHUNK:(s + 1) * CHUNK]
        order = np.argsort(ys == 0, kind="stable")  # nonzero first
        perm[s * CHUNK:(s + 1) * CHUNK] = s * CHUNK + order
        ks.append(int((ys != 0).sum()))
    catp = np.asarray(cat)[perm]
    catT = np.ascontiguousarray(catp.T).astype(ml_dtypes.bfloat16)
    xT = np.ascontiguousarray(np.asarray(batch_x).T).astype(ml_dtypes.bfloat16)
    return catT, xT, ks


def host_epilogue(results, batch_x, phi, bias):
    """results: list over cores of {'w_out': [128,16], 'r_out': [128,16]}."""
    theta = np.exp(np.asarray(batch_x, np.float64) @ np.asarray(phi, np.float64).T)
    out = np.empty(B, np.float64)
    for c in range(NCORES):
        w = np.asarray(results[c]["w_out"], np.float64)
        f = w + np.asarray(results[c]["r_out"], np.float64)
        for bt in range(NBT):
            cols = slice(bt * S, (bt + 1) * S)
            z = f[:, cols].sum(axis=1)
            bidx = c * BL + bt * P + np.arange(P)
            out[bidx] = ((w[:, cols] / z[:, None]) * theta[bidx, :]).sum(axis=1)
    out = out + float(np.asarray(bias).reshape(-1)[0])
    return (1.0 / (1.0 + np.exp(-out))).astype(np.float32)


def make_in_maps(catT, xT):
    return [
        {
            "catTp": catT,
            "xTl": np.ascontiguousarray(xT[:, c * BL:(c + 1) * BL]),
        }
        for c in range(NCORES)
    ]


def kernel(batch_x, cat, y, phi, bias):
    catT, xT, ks = host_prep(batch_x, cat, y)
    nc = build_program(ks)
    res = bass_utils.run_bass_kernel_spmd(nc, make_in_maps(catT, xT), core_ids=list(range(NCORES)))
    return host_epilogue(res.results, batch_x, phi, bias)
